# revision 49
# baseline (speedup 1.0000x reference)
"""CfC (nn_Actor) Trainium2 kernel: 8-core data-parallel recurrent scan.

Layout strategy: everything "transposed" (features on SBUF partitions, batch on
the free dim) so the recurrent state hT feeds the next step's matmuls with zero
data movement. Host-side numpy does all the layout marshalling: x is
pre-transposed with a ones-row appended (folds the backbone bias into the
matmul), weights are pre-transposed/pre-scaled (lecun_tanh scales folded into
weights; sigmoid computed as 0.5*tanh(0.5x)+0.5 with the 0.5 folded in).

ILV independent half-batch scans are interleaved so one scan's matmuls fill
the other's activation/vector-engine bubbles (the step chain is serial).
Per-feature biases enter PSUM via small indicator matmuls packed into
concurrent PE row-groups, scheduled off the critical chain.
"""

import numpy as np
import ml_dtypes

import concourse.bass as bass
import concourse.tile as tile
from concourse import bacc, mybir
from concourse.bass_utils import run_bass_kernel_spmd

BF16 = mybir.dt.bfloat16
F32 = mybir.dt.float32
F8 = mybir.dt.float8e4
TANH = mybir.ActivationFunctionType.Tanh
SIGMOID = mybir.ActivationFunctionType.Sigmoid
ALU = mybir.AluOpType

B, T, D, H, BBD, A = 1024, 256, 64, 512, 128, 8
NCORES = 8
BC = B // NCORES  # batch per core = 128
NK = H // 128  # 4 h-tiles

ILV = 2  # interleaved independent scans per core
RING_EDGES = True  # pin Scalar-engine ring order: big_A, g_B, g_A(t+1), big_B
FP8_BIAS = False  # fp8+DoubleRow biases: no measured speedup (chain-bound), costs accuracy
W = BC // ILV  # batch width per scan
WARMUP_MMS = 0  # PE warm state does not persist for this duty cycle; skip

S_IN = 0.666
S_OUT = 1.7159

_CACHE = {}


def _emit_scan(nc, tc, pools, consts, s):
    """Emit one independent scan over T steps for batch slice s (width W).

    PSUM layout per step (one [128, 1024] tile = 2 banks, double-buffered):
      bank0 (cols 0:512)    = ff1 (4 blocks of W) | ff2 (4 blocks of W)
      bank1 (cols 512:1024) = tau (4 blocks of W) at 512:768, gpre at 768:832
    The two bias matmuls are the per-bank `start=True` cleaners; the gpre
    group writes into bank1's cleared-but-unwritten columns (start=False
    fresh-writes where has_written is unset).
    """
    gpool, ffvpool, hpool, wpool, psg_pool, psf_pool = pools
    xT, wbbh, wbbx, wff, biasc, indc, wf1, bf1T, wf2, bf2, ones = consts[:11]
    wbbhh = consts[11]  # 0.5 * wbbh, for the v-part of the distributed gpre

    GP = 768  # gpre column offset inside the merged PSUM tile
    ff1_prev = None
    v_prev = None
    for t in range(T):
        psf = psf_pool.tile([128, 1024], F32, tag=f"psf{s}")
        # bias writes / bank cleaners (off the critical chain). Two matmuls
        # in distinct PE row-groups (rows 0-7 / 64-67) targeting different
        # banks run concurrently in the array. With FP8_BIAS, fp8e4 operands
        # + DoubleRow pack K-row pairs -> 0.5 cycles/output-row (half time).
        if FP8_BIAS:
            b8, i8 = consts[12], consts[13]
            DR = mybir.MatmulPerfMode.DoubleRow
            nc.tensor.matmul(
                psf[:, 0 : 8 * W],
                b8[0:4, :].rearrange("k (a m) -> k a m", a=2),
                i8[0:4, 0 : 16 * W].rearrange("k (a n) -> k a n", a=2),
                start=True, stop=False, skip_group_check=True, perf_mode=DR,
            )
            # tau bias stays bf16: at N=256 DoubleRow's doubled LDWEIGHTS
            # outweighs the halved streaming (measured 388 -> 600ns)
            taubias = nc.tensor.matmul(
                psf[:, 8 * W : 12 * W], biasc[64:68, :], indc[64:68, 0 : 4 * W],
                start=True, stop=False, skip_group_check=True,
            )
        else:
            nc.tensor.matmul(
                psf[:, 0 : 8 * W], biasc[0:8, :], indc[0:8, 0 : 8 * W],
                start=True, stop=False, skip_group_check=True,
            )
            taubias = nc.tensor.matmul(
                psf[:, 8 * W : 12 * W], biasc[64:68, :], indc[64:68, 0 : 4 * W],
                start=True, stop=False, skip_group_check=True,
            )

        # ---- backbone pre-activation, transposed: [BBD=128, W] ----
        xmm = nc.tensor.matmul(
            psf[:, GP : GP + W], wbbx[:],
            xT[:, t * BC + s * W : t * BC + (s + 1) * W],
            start=False, stop=(t == 0), skip_group_check=True,
        )
        # bank1's cleaner (taubias) must precede the gpre writers
        tile.add_dep_helper(xmm.ins, taubias.ins, sync=False, reason="bank1 clear order")
        if t > 0:
            # gpre = Wbbh@h distributed through h = ff1 + 0.5*v:
            # ff1-part matmuls are ready right after the previous bigACT
            # (they overlap the DVE combine); only the v-part gates on DVE.
            for k in range(NK):
                nc.tensor.matmul(
                    psf[:, GP : GP + W],
                    wbbh[:, k * 128 : (k + 1) * 128],
                    ff1_prev[:, k * W : (k + 1) * W],
                    start=False, stop=False, skip_group_check=True,
                )
            for k in range(NK):
                nc.tensor.matmul(
                    psf[:, GP : GP + W],
                    wbbhh[:, k * 128 : (k + 1) * 128],
                    v_prev[:, k * W : (k + 1) * W],
                    start=False, stop=(k == NK - 1), skip_group_check=True,
                )
        gT = gpool.tile([128, W], BF16, tag=f"gT{s}")
        gact = nc.scalar.activation(gT[:], psf[:, GP : GP + W], TANH)

        # ---- ff1 | ff2 | tau data matmuls ----
        for j in range(3):
            for k in range(NK):
                c0 = (j * NK + k) * W
                nc.tensor.matmul(
                    psf[:, c0 : c0 + W],
                    wff[:, (j * NK + k) * 128 : (j * NK + k + 1) * 128],
                    gT[:],
                    start=False,
                    stop=(j == 2 and k == NK - 1), skip_group_check=True,
                )
        ffv = ffvpool.tile([128, 12 * W], BF16, tag=f"ffv{s}")
        bigact = nc.scalar.activation(ffv[:], psf[:, 0 : 12 * W], TANH)
        ff1v = ffv[:, 0 : 4 * W]
        ff2v = ffv[:, 4 * W : 8 * W]
        tauv = ffv[:, 8 * W : 12 * W]

        # ---- combine: h = ff1 + sig*(ff2-ff1), sig = (tau+1)/2 ----
        # fused: u = ff2-ff1; v = (tau+1)*u; h = 0.5*v + ff1
        u = wpool.tile([128, 4 * W], BF16, tag=f"u{s}")
        uop = nc.vector.tensor_tensor(u[:], ff2v, ff1v, ALU.subtract)
        v = hpool.tile([128, 4 * W], BF16, tag=f"v{s}")
        vop = nc.vector.scalar_tensor_tensor(v[:], tauv, 1.0, u[:], ALU.add, ALU.mult)
        ff1_prev = ff1v
        v_prev = v
        yield (gact, bigact, uop, vop)  # step boundary: interleave scans here

    # ---- head: materialize h = ff1 + 0.5*v once, then y1 = tanh(Wf1@h+bf1)
    h_prev = wpool.tile([128, 4 * W], BF16, tag=f"hl{s}")
    nc.vector.scalar_tensor_tensor(h_prev[:], v_prev[:], 0.5, ff1_prev, ALU.mult, ALU.add)
    psy = psf_pool.tile([128, 4 * W], F32, tag=f"psf{s}")
    nc.tensor.matmul(
        psy[:, 0 : 4 * W], bf1T[0:4, :], indc[0:4, 0 : 4 * W],
        start=True, stop=False,
    )
    for m in range(4):
        for k in range(NK):
            nc.tensor.matmul(
                psy[:, m * W : (m + 1) * W],
                wf1[:, (m * NK + k) * 128 : (m * NK + k + 1) * 128],
                h_prev[:, k * W : (k + 1) * W],
                start=False, stop=(m == 3 and k == NK - 1),
            )
    y1v = ffvpool.tile([128, 4 * W], BF16, tag=f"ffv{s}")
    nc.scalar.activation(y1v[:], psy[:], TANH)

    # ---- head 2: out = tanh(Wf2 @ y1 + bf2) : [A=8, W] ----
    pso = psf_pool.tile([128, W], F32, tag=f"psf{s}")
    nc.tensor.matmul(pso[0:A, :], bf2[0:1, 0:A], ones[0:1, 0:W], start=True, stop=False)
    for k in range(NK):
        nc.tensor.matmul(
            pso[0:A, :],
            wf2[:, k * A : (k + 1) * A],
            y1v[:, k * W : (k + 1) * W],
            start=False, stop=(k == NK - 1),
        )
    out_sb = wpool.tile([128, W], F32, tag=f"osb{s}")
    nc.scalar.activation(out_sb[0:A, :], pso[0:A, :], TANH)
    return out_sb


def _build_program():
    nc = bacc.Bacc("TRN2", target_bir_lowering=False, debug=False)

    xT_e = nc.declare_dram_parameter("xT", [D + 1, T * BC], BF16, isOutput=False)
    wbbh_e = nc.declare_dram_parameter("wbbh", [128, NK * 128], BF16, isOutput=False)
    wbbx_e = nc.declare_dram_parameter("wbbx", [D + 1, 128], BF16, isOutput=False)
    wff_e = nc.declare_dram_parameter("wff", [128, 12 * 128], BF16, isOutput=False)
    biasc_e = nc.declare_dram_parameter("biasc", [68, 128], BF16, isOutput=False)
    indc_e = nc.declare_dram_parameter("indc", [68, 8 * W], BF16, isOutput=False)
    wf1_e = nc.declare_dram_parameter("wf1", [128, 16 * 128], BF16, isOutput=False)
    bf1T_e = nc.declare_dram_parameter("bf1T", [4, 128], BF16, isOutput=False)
    wf2_e = nc.declare_dram_parameter("wf2", [128, NK * A], BF16, isOutput=False)
    bf2_e = nc.declare_dram_parameter("bf2", [1, A], BF16, isOutput=False)
    ones_e = nc.declare_dram_parameter("ones", [1, BC], BF16, isOutput=False)
    wbbhh_e = nc.declare_dram_parameter("wbbhh", [128, NK * 128], BF16, isOutput=False)
    b8_e = nc.declare_dram_parameter("b8", [68, 256], F8, isOutput=False)
    i8_e = nc.declare_dram_parameter("i8", [68, 1024], F8, isOutput=False)
    out_e = nc.declare_dram_parameter("out", [A, BC], F32, isOutput=True)

    from contextlib import ExitStack

    with tile.TileContext(nc) as tc, ExitStack() as ctx:
        const = ctx.enter_context(tc.tile_pool(name="const", bufs=1))
        gpool = ctx.enter_context(tc.tile_pool(name="g", bufs=4))
        ffvpool = ctx.enter_context(tc.tile_pool(name="ffv", bufs=4))
        hpool = ctx.enter_context(tc.tile_pool(name="h", bufs=4))
        wpool = ctx.enter_context(tc.tile_pool(name="work", bufs=4))
        psg_pool = None
        psf_pool = ctx.enter_context(tc.tile_pool(name="psf", bufs=2, space="PSUM"))

        def cload(ext, shape, tag, dtype=BF16):
            t = const.tile(shape, dtype, tag=tag)
            nc.sync.dma_start(t[:], ext[:])
            return t

        consts = (
            cload(xT_e, [D + 1, T * BC], "xT"),
            cload(wbbh_e, [128, NK * 128], "wbbh"),
            cload(wbbx_e, [D + 1, 128], "wbbx"),
            cload(wff_e, [128, 12 * 128], "wff"),
            cload(biasc_e, [68, 128], "biasc"),
            cload(indc_e, [68, 8 * W], "indc"),
            cload(wf1_e, [128, 16 * 128], "wf1"),
            cload(bf1T_e, [4, 128], "bf1T"),
            cload(wf2_e, [128, NK * A], "wf2"),
            cload(bf2_e, [1, A], "bf2"),
            cload(ones_e, [1, BC], "ones"),
            cload(wbbhh_e, [128, NK * 128], "wbbhh"),
            cload(b8_e, [68, 256], "b8", dtype=F8),
            cload(i8_e, [68, 1024], "i8", dtype=F8),
        )
        pools = (gpool, ffvpool, hpool, wpool, psg_pool, psf_pool)

        # PE clock warmup: HAM un-throttles (1.2 -> 2.4 GHz) only after a
        # sustained-busy window; run garbage matmuls while the xT DMA streams.
        wff_c = consts[3]
        warm_ps = psf_pool.tile([128, 512], F32, tag="psf0")
        for _ in range(WARMUP_MMS):
            nc.tensor.matmul(warm_ps[:], wff_c[:, 0:128], wff_c[:, 0:512],
                             start=True, stop=True)

        # Emit the two scans step-interleaved, and pin the Scalar-engine order
        # per period to [big_A, g_B, g_A, big_B]: this keeps each scan's
        # ff-matmul latency and the other scan's bigACT off the critical ring.
        gens = [_emit_scan(nc, tc, pools, consts, s) for s in range(ILV)]
        outs = [None] * ILV
        done = 0
        prev_bB = None
        while done < ILV:
            acts = []
            for s, g in enumerate(gens):
                if outs[s] is None:
                    try:
                        acts.append(next(g))
                    except StopIteration as e:
                        outs[s] = e.value
                        acts.append(None)
                        done += 1
            # Scalar ring per period: big_A(t), g_B(t), g_A(t+1), big_B(t)
            if RING_EDGES and ILV == 2 and len(acts) == 2 and acts[0] and acts[1]:
                (gA, bA, uA, hA), (gB, bB, uB, hB) = acts
                if prev_bB is not None:
                    prev_gB, prev_bB_i, prev_hB = prev_bB
                    tile.add_dep_helper(gA.ins, prev_gB.ins, sync=False,
                                        reason="scalar ring g_A after g_B")
                    tile.add_dep_helper(prev_bB_i.ins, gA.ins, sync=False,
                                        reason="scalar ring big_B after g_A")
                    tile.add_dep_helper(bA.ins, prev_bB_i.ins, sync=False,
                                        reason="scalar ring big_A after big_B")
                    # DVE ring: u_A(t) after h_B(t-1), u_B(t) after h_A(t)
                    tile.add_dep_helper(uA.ins, prev_hB.ins, sync=False,
                                        reason="dve ring u_A after h_B")
                tile.add_dep_helper(gB.ins, bA.ins, sync=False,
                                    reason="scalar ring g_B after big_A")
                tile.add_dep_helper(uB.ins, hA.ins, sync=False,
                                    reason="dve ring u_B after h_A")
                prev_bB = (gB, bB, hB)
        for s, osb in enumerate(outs):
            nc.sync.dma_start(out_e[:, s * W : (s + 1) * W], osb[0:A, :])

    nc.compile()
    return nc


def _to_bf16(x):
    return np.ascontiguousarray(x.astype(ml_dtypes.bfloat16))


def _prep_shared(Wbb, bbb, Wff1, bff1, Wff2, bff2, Wta, bta, Wtb, btb, Wf1, bf1, Wf2, bf2):
    Wbbx = Wbb[:, :D]  # [128, 64]
    Wbbh = Wbb[:, D:]  # [128, 512]

    wbbx_aug = np.zeros((D + 1, 128), np.float32)
    wbbx_aug[:D, :] = (S_IN * Wbbx).T
    wbbx_aug[D, :] = S_IN * bbb

    wbbh_arr = np.zeros((128, NK * 128), np.float32)
    for k in range(NK):
        wbbh_arr[:, k * 128 : (k + 1) * 128] = (S_IN * Wbbh[:, k * 128 : (k + 1) * 128]).T

    Aj = [S_OUT * Wff1, S_OUT * Wff2, 0.5 * S_OUT * (Wta + Wtb)]  # each [512, 128]
    bj = [bff1, bff2, 0.5 * (bta + btb)]
    wff_arr = np.zeros((128, 12 * 128), np.float32)
    for j in range(3):
        for k in range(NK):
            wff_arr[:, (j * NK + k) * 128 : (j * NK + k + 1) * 128] = Aj[j][
                k * 128 : (k + 1) * 128, :
            ].T

    # bias pack: rows 0-7 = [bff1 k-slices, bff2 k-slices] (bank0, row-group
    # 0), rows 64-67 = tau bias k-slices (bank1, row-group 2).
    biasc_arr = np.zeros((68, 128), np.float32)
    biasc_arr[0:4, :] = bj[0].reshape(4, 128)
    biasc_arr[4:8, :] = bj[1].reshape(4, 128)
    biasc_arr[64:68, :] = bj[2].reshape(4, 128)

    # indicator: rows 0-7 select W-col blocks 0..7 (8W); rows 64-67 select
    # blocks 0..3 (4W). Rows 0-3 double as the y1-head bias indicator.
    indc_arr = np.zeros((68, 8 * W), np.float32)
    for i in range(8):
        indc_arr[i, i * W : (i + 1) * W] = 1.0
    for i in range(4):
        indc_arr[64 + i, i * W : (i + 1) * W] = 1.0

    # fp8 DoubleRow packs: K-row pairs (kp, j) -> virtual row 2*kp+j
    b8_arr = np.zeros((68, 256), np.float32)
    for kp in range(4):
        for j in range(2):
            b8_arr[kp, j * 128 : (j + 1) * 128] = biasc_arr[2 * kp + j]
    for kp in range(2):
        for j in range(2):
            b8_arr[64 + kp, j * 128 : (j + 1) * 128] = biasc_arr[64 + 2 * kp + j]
    i8_arr = np.zeros((68, 1024), np.float32)
    for kp in range(4):
        for j in range(2):
            i8_arr[kp, j * 512 : (j + 1) * 512] = indc_arr[2 * kp + j]
    for kp in range(2):
        for j in range(2):
            i8_arr[64 + kp, j * 256 : (j + 1) * 256] = indc_arr[64 + 2 * kp + j, 0:256]

    wf1_arr = np.zeros((128, 16 * 128), np.float32)
    for m in range(4):
        for k in range(NK):
            wf1_arr[:, (m * NK + k) * 128 : (m * NK + k + 1) * 128] = Wf1[
                m * 128 : (m + 1) * 128, k * 128 : (k + 1) * 128
            ].T
    bf1T_arr = bf1.reshape(4, 128)

    wf2_arr = np.zeros((128, NK * A), np.float32)
    for k in range(NK):
        wf2_arr[:, k * A : (k + 1) * A] = Wf2[:, k * 128 : (k + 1) * 128].T
    bf2_arr = bf2.reshape(1, A)
    ones_arr = np.ones((1, BC), np.float32)

    return {
        "wbbh": _to_bf16(wbbh_arr),
        "wbbhh": _to_bf16(0.5 * wbbh_arr),
        "wbbx": _to_bf16(wbbx_aug),
        "wff": _to_bf16(wff_arr),
        "biasc": _to_bf16(biasc_arr),
        "indc": _to_bf16(indc_arr),
        "wf1": _to_bf16(wf1_arr),
        "bf1T": _to_bf16(bf1T_arr),
        "wf2": _to_bf16(wf2_arr),
        "bf2": _to_bf16(bf2_arr),
        "ones": _to_bf16(ones_arr),
        "b8": np.ascontiguousarray(b8_arr.astype(ml_dtypes.float8_e4m3)),
        "i8": np.ascontiguousarray(i8_arr.astype(ml_dtypes.float8_e4m3)),
    }


def _prep_core_x(state_c):
    # state_c: [BC, T, D] -> xT_aug [D+1, T*BC] with ones row
    xT = state_c.transpose(2, 1, 0).reshape(D, T * BC)  # [d, t*BC+b]
    xa = np.ones((D + 1, T * BC), np.float32)
    xa[:D, :] = xT
    return _to_bf16(xa)


def _get_program():
    if "nc" not in _CACHE:
        _CACHE["nc"] = _build_program()
    return _CACHE["nc"]


def run(inputs, trace=False, trace_kwargs=None):
    inputs = {k: np.asarray(v) for k, v in inputs.items()}
    nc = _get_program()
    shared = _prep_shared(
        inputs["Wbb"], inputs["bbb"], inputs["Wff1"], inputs["bff1"],
        inputs["Wff2"], inputs["bff2"], inputs["Wta"], inputs["bta"],
        inputs["Wtb"], inputs["btb"], inputs["Wf1"], inputs["bf1"],
        inputs["Wf2"], inputs["bf2"],
    )
    state = inputs["state"].astype(np.float32)
    in_maps = []
    for c in range(NCORES):
        m = dict(shared)
        m["xT"] = _prep_core_x(state[c * BC : (c + 1) * BC])
        in_maps.append(m)
    res = run_bass_kernel_spmd(
        nc, in_maps, core_ids=list(range(NCORES)), trace=trace,
        **(trace_kwargs or {}),
    )
    out = np.concatenate(
        [np.asarray(res.results[c]["out"]).T for c in range(NCORES)], axis=0
    )
    return out.astype(np.float32), res


def kernel(**inputs):
    out, _ = run(inputs, trace=False)
    return out


# revision 50
# speedup vs baseline: 1.1810x; 1.1810x over previous
"""CfC (nn_Actor) Trainium2 kernel: 8-core data-parallel recurrent scan.

Layout strategy: everything "transposed" (features on SBUF partitions, batch on
the free dim) so the recurrent state hT feeds the next step's matmuls with zero
data movement. Host-side numpy does all the layout marshalling: x is
pre-transposed with a ones-row appended (folds the backbone bias into the
matmul), weights are pre-transposed/pre-scaled (lecun_tanh scales folded into
weights; sigmoid computed as 0.5*tanh(0.5x)+0.5 with the 0.5 folded in).

ILV independent half-batch scans are interleaved so one scan's matmuls fill
the other's activation/vector-engine bubbles (the step chain is serial).
Per-feature biases enter PSUM via small indicator matmuls packed into
concurrent PE row-groups, scheduled off the critical chain.
"""

import numpy as np
import ml_dtypes

import concourse.bass as bass
import concourse.tile as tile
from concourse import bacc, mybir
from concourse.bass_utils import run_bass_kernel_spmd

BF16 = mybir.dt.bfloat16
F32 = mybir.dt.float32
F8 = mybir.dt.float8e4
TANH = mybir.ActivationFunctionType.Tanh
SIGMOID = mybir.ActivationFunctionType.Sigmoid
ALU = mybir.AluOpType

B, T, D, H, BBD, A = 1024, 256, 64, 512, 128, 8
NCORES = 8
BC = B // NCORES  # batch per core = 128
NK = H // 128  # 4 h-tiles

ILV = 2  # interleaved independent scans per core
RING_EDGES = True  # pin Scalar-engine ring order: big_A, g_B, g_A(t+1), big_B
FP8_BIAS = False  # fp8+DoubleRow biases: no measured speedup (chain-bound), costs accuracy
W = BC // ILV  # batch width per scan
WARMUP_MMS = 0  # PE warm state does not persist for this duty cycle; skip

S_IN = 0.666
S_OUT = 1.7159

_CACHE = {}


def _emit_scan(nc, tc, pools, consts, s):
    """Emit one independent scan over T steps for batch slice s (width W).

    PSUM layout per step (one [128, 1024] tile = 2 banks, double-buffered):
      bank0 (cols 0:512)    = ff1 (4 blocks of W) | ff2 (4 blocks of W)
      bank1 (cols 512:1024) = tau (4 blocks of W) at 512:768, gpre at 768:832
    The two bias matmuls are the per-bank `start=True` cleaners; the gpre
    group writes into bank1's cleared-but-unwritten columns (start=False
    fresh-writes where has_written is unset).
    """
    gpool, ffvpool, hpool, wpool, psg_pool, psf_pool = pools
    xT, wbbh, wbbx, wff, biasc, indc, wf1, bf1T, wf2, bf2, ones = consts[:11]
    wbbhh = consts[11]  # 0.5 * wbbh, for the v-part of the distributed gpre

    GP = 768  # gpre column offset inside the merged PSUM tile
    ff1_prev = None
    v_prev = None
    for t in range(T):
        psf = psf_pool.tile([128, 1024], F32, tag=f"psf{s}")
        # bias writes / bank cleaners (off the critical chain). Two matmuls
        # in distinct PE row-groups (rows 0-7 / 64-67) targeting different
        # banks run concurrently in the array. With FP8_BIAS, fp8e4 operands
        # + DoubleRow pack K-row pairs -> 0.5 cycles/output-row (half time).
        if FP8_BIAS:
            b8, i8 = consts[12], consts[13]
            DR = mybir.MatmulPerfMode.DoubleRow
            nc.tensor.matmul(
                psf[:, 0 : 8 * W],
                b8[0:4, :].rearrange("k (a m) -> k a m", a=2),
                i8[0:4, 0 : 16 * W].rearrange("k (a n) -> k a n", a=2),
                start=True, stop=False, skip_group_check=True, perf_mode=DR,
            )
            # tau bias stays bf16: at N=256 DoubleRow's doubled LDWEIGHTS
            # outweighs the halved streaming (measured 388 -> 600ns)
            taubias = nc.tensor.matmul(
                psf[:, 8 * W : 12 * W], biasc[64:68, :], indc[64:68, 0 : 4 * W],
                start=True, stop=False, skip_group_check=True,
            )
        else:
            nc.tensor.matmul(
                psf[:, 0 : 8 * W], biasc[0:8, :], indc[0:8, 0 : 8 * W],
                start=True, stop=False, skip_group_check=True,
            )
            taubias = nc.tensor.matmul(
                psf[:, 8 * W : 12 * W], biasc[64:68, :], indc[64:68, 0 : 4 * W],
                start=True, stop=False, skip_group_check=True,
            )

        # ---- backbone pre-activation, transposed: [BBD=128, W] ----
        xmm = nc.tensor.matmul(
            psf[:, GP : GP + W], wbbx[:],
            xT[:, t * BC + s * W : t * BC + (s + 1) * W],
            start=False, stop=(t == 0), skip_group_check=True,
        )
        # bank1's cleaner (taubias) must precede the gpre writers
        tile.add_dep_helper(xmm.ins, taubias.ins, sync=False, reason="bank1 clear order")
        if t > 0:
            # gpre = Wbbh@h distributed through h = ff1 + 0.5*v:
            # ff1-part matmuls are ready right after the previous bigACT
            # (they overlap the DVE combine); only the v-part gates on DVE.
            for k in range(NK):
                nc.tensor.matmul(
                    psf[:, GP : GP + W],
                    wbbh[:, k * 128 : (k + 1) * 128],
                    ff1_prev[:, k * W : (k + 1) * W],
                    start=False, stop=False, skip_group_check=True,
                )
            for k in range(NK):
                nc.tensor.matmul(
                    psf[:, GP : GP + W],
                    wbbhh[:, k * 128 : (k + 1) * 128],
                    v_prev[:, k * W : (k + 1) * W],
                    start=False, stop=(k == NK - 1), skip_group_check=True,
                )
        gT = gpool.tile([128, W], BF16, tag=f"gT{s}")
        gact = nc.scalar.activation(gT[:], psf[:, GP : GP + W], TANH)

        # ---- ff1 | ff2 | tau data matmuls ----
        for j in range(3):
            for k in range(NK):
                c0 = (j * NK + k) * W
                nc.tensor.matmul(
                    psf[:, c0 : c0 + W],
                    wff[:, (j * NK + k) * 128 : (j * NK + k + 1) * 128],
                    gT[:],
                    start=False,
                    stop=(j == 2 and k == NK - 1), skip_group_check=True,
                )
        ffv = ffvpool.tile([128, 12 * W], BF16, tag=f"ffv{s}")
        bigact = nc.scalar.activation(ffv[:], psf[:, 0 : 12 * W], TANH)
        ff1v = ffv[:, 0 : 4 * W]
        ff2v = ffv[:, 4 * W : 8 * W]
        tauv = ffv[:, 8 * W : 12 * W]

        # ---- combine: h = ff1 + sig*(ff2-ff1), sig = (tau+1)/2 ----
        # fused: u = ff2-ff1; v = (tau+1)*u; h = 0.5*v + ff1
        u = wpool.tile([128, 4 * W], BF16, tag=f"u{s}")
        uop = nc.vector.tensor_tensor(u[:], ff2v, ff1v, ALU.subtract)
        v = hpool.tile([128, 4 * W], BF16, tag=f"v{s}")
        vop = nc.vector.scalar_tensor_tensor(v[:], tauv, 1.0, u[:], ALU.add, ALU.mult)
        ff1_prev = ff1v
        v_prev = v
        yield (gact, bigact, uop, vop)  # step boundary: interleave scans here

    # ---- head: materialize h = ff1 + 0.5*v once, then y1 = tanh(Wf1@h+bf1)
    h_prev = wpool.tile([128, 4 * W], BF16, tag=f"hl{s}")
    nc.vector.scalar_tensor_tensor(h_prev[:], v_prev[:], 0.5, ff1_prev, ALU.mult, ALU.add)
    psy = psf_pool.tile([128, 4 * W], F32, tag=f"psf{s}")
    nc.tensor.matmul(
        psy[:, 0 : 4 * W], bf1T[0:4, :], indc[0:4, 0 : 4 * W],
        start=True, stop=False,
    )
    for m in range(4):
        for k in range(NK):
            nc.tensor.matmul(
                psy[:, m * W : (m + 1) * W],
                wf1[:, (m * NK + k) * 128 : (m * NK + k + 1) * 128],
                h_prev[:, k * W : (k + 1) * W],
                start=False, stop=(m == 3 and k == NK - 1),
            )
    y1v = ffvpool.tile([128, 4 * W], BF16, tag=f"ffv{s}")
    nc.scalar.activation(y1v[:], psy[:], TANH)

    # ---- head 2: out = tanh(Wf2 @ y1 + bf2) : [A=8, W] ----
    pso = psf_pool.tile([128, W], F32, tag=f"psf{s}")
    nc.tensor.matmul(pso[0:A, :], bf2[0:1, 0:A], ones[0:1, 0:W], start=True, stop=False)
    for k in range(NK):
        nc.tensor.matmul(
            pso[0:A, :],
            wf2[:, k * A : (k + 1) * A],
            y1v[:, k * W : (k + 1) * W],
            start=False, stop=(k == NK - 1),
        )
    out_sb = wpool.tile([128, W], F32, tag=f"osb{s}")
    nc.scalar.activation(out_sb[0:A, :], pso[0:A, :], TANH)
    return out_sb


def _build_program():
    nc = bacc.Bacc("TRN2", target_bir_lowering=False, debug=False)

    xT_e = nc.declare_dram_parameter("xT", [D + 1, T * BC], BF16, isOutput=False)
    wbbh_e = nc.declare_dram_parameter("wbbh", [128, NK * 128], BF16, isOutput=False)
    wbbx_e = nc.declare_dram_parameter("wbbx", [D + 1, 128], BF16, isOutput=False)
    wff_e = nc.declare_dram_parameter("wff", [128, 12 * 128], BF16, isOutput=False)
    biasc_e = nc.declare_dram_parameter("biasc", [68, 128], BF16, isOutput=False)
    indc_e = nc.declare_dram_parameter("indc", [68, 8 * W], BF16, isOutput=False)
    wf1_e = nc.declare_dram_parameter("wf1", [128, 16 * 128], BF16, isOutput=False)
    bf1T_e = nc.declare_dram_parameter("bf1T", [4, 128], BF16, isOutput=False)
    wf2_e = nc.declare_dram_parameter("wf2", [128, NK * A], BF16, isOutput=False)
    bf2_e = nc.declare_dram_parameter("bf2", [1, A], BF16, isOutput=False)
    ones_e = nc.declare_dram_parameter("ones", [1, BC], BF16, isOutput=False)
    wbbhh_e = nc.declare_dram_parameter("wbbhh", [128, NK * 128], BF16, isOutput=False)
    b8_e = nc.declare_dram_parameter("b8", [68, 256], F8, isOutput=False)
    i8_e = nc.declare_dram_parameter("i8", [68, 1024], F8, isOutput=False)
    out_e = nc.declare_dram_parameter("out", [A, BC], F32, isOutput=True)

    from contextlib import ExitStack

    with tile.TileContext(nc) as tc, ExitStack() as ctx:
        const = ctx.enter_context(tc.tile_pool(name="const", bufs=1))
        gpool = ctx.enter_context(tc.tile_pool(name="g", bufs=4))
        ffvpool = ctx.enter_context(tc.tile_pool(name="ffv", bufs=4))
        hpool = ctx.enter_context(tc.tile_pool(name="h", bufs=6))
        wpool = ctx.enter_context(tc.tile_pool(name="work", bufs=6))
        psg_pool = None
        psf_pool = ctx.enter_context(tc.tile_pool(name="psf", bufs=2, space="PSUM"))

        def cload(ext, shape, tag, dtype=BF16):
            t = const.tile(shape, dtype, tag=tag)
            nc.sync.dma_start(t[:], ext[:])
            return t

        consts = (
            cload(xT_e, [D + 1, T * BC], "xT"),
            cload(wbbh_e, [128, NK * 128], "wbbh"),
            cload(wbbx_e, [D + 1, 128], "wbbx"),
            cload(wff_e, [128, 12 * 128], "wff"),
            cload(biasc_e, [68, 128], "biasc"),
            cload(indc_e, [68, 8 * W], "indc"),
            cload(wf1_e, [128, 16 * 128], "wf1"),
            cload(bf1T_e, [4, 128], "bf1T"),
            cload(wf2_e, [128, NK * A], "wf2"),
            cload(bf2_e, [1, A], "bf2"),
            cload(ones_e, [1, BC], "ones"),
            cload(wbbhh_e, [128, NK * 128], "wbbhh"),
            cload(b8_e, [68, 256], "b8", dtype=F8),
            cload(i8_e, [68, 1024], "i8", dtype=F8),
        )
        pools = (gpool, ffvpool, hpool, wpool, psg_pool, psf_pool)

        # PE clock warmup: HAM un-throttles (1.2 -> 2.4 GHz) only after a
        # sustained-busy window; run garbage matmuls while the xT DMA streams.
        wff_c = consts[3]
        warm_ps = psf_pool.tile([128, 512], F32, tag="psf0")
        for _ in range(WARMUP_MMS):
            nc.tensor.matmul(warm_ps[:], wff_c[:, 0:128], wff_c[:, 0:512],
                             start=True, stop=True)

        # Emit the two scans step-interleaved, and pin the Scalar-engine order
        # per period to [big_A, g_B, g_A, big_B]: this keeps each scan's
        # ff-matmul latency and the other scan's bigACT off the critical ring.
        gens = [_emit_scan(nc, tc, pools, consts, s) for s in range(ILV)]
        outs = [None] * ILV
        done = 0
        prev_bB = None
        while done < ILV:
            acts = []
            for s, g in enumerate(gens):
                if outs[s] is None:
                    try:
                        acts.append(next(g))
                    except StopIteration as e:
                        outs[s] = e.value
                        acts.append(None)
                        done += 1
            # Scalar ring per period: big_A(t), g_B(t), g_A(t+1), big_B(t)
            if RING_EDGES and ILV == 2 and len(acts) == 2 and acts[0] and acts[1]:
                (gA, bA, uA, hA), (gB, bB, uB, hB) = acts
                if prev_bB is not None:
                    prev_gB, prev_bB_i, prev_hB = prev_bB
                    tile.add_dep_helper(gA.ins, prev_gB.ins, sync=False,
                                        reason="scalar ring g_A after g_B")
                    tile.add_dep_helper(prev_bB_i.ins, gA.ins, sync=False,
                                        reason="scalar ring big_B after g_A")
                    tile.add_dep_helper(bA.ins, prev_bB_i.ins, sync=False,
                                        reason="scalar ring big_A after big_B")
                    # DVE ring: u_A(t) after h_B(t-1), u_B(t) after h_A(t)
                    tile.add_dep_helper(uA.ins, prev_hB.ins, sync=False,
                                        reason="dve ring u_A after h_B")
                tile.add_dep_helper(gB.ins, bA.ins, sync=False,
                                    reason="scalar ring g_B after big_A")
                tile.add_dep_helper(uB.ins, hA.ins, sync=False,
                                    reason="dve ring u_B after h_A")
                prev_bB = (gB, bB, hB)
        for s, osb in enumerate(outs):
            nc.sync.dma_start(out_e[:, s * W : (s + 1) * W], osb[0:A, :])

    nc.compile()
    return nc


def _to_bf16(x):
    return np.ascontiguousarray(x.astype(ml_dtypes.bfloat16))


def _prep_shared(Wbb, bbb, Wff1, bff1, Wff2, bff2, Wta, bta, Wtb, btb, Wf1, bf1, Wf2, bf2):
    Wbbx = Wbb[:, :D]  # [128, 64]
    Wbbh = Wbb[:, D:]  # [128, 512]

    wbbx_aug = np.zeros((D + 1, 128), np.float32)
    wbbx_aug[:D, :] = (S_IN * Wbbx).T
    wbbx_aug[D, :] = S_IN * bbb

    wbbh_arr = np.zeros((128, NK * 128), np.float32)
    for k in range(NK):
        wbbh_arr[:, k * 128 : (k + 1) * 128] = (S_IN * Wbbh[:, k * 128 : (k + 1) * 128]).T

    Aj = [S_OUT * Wff1, S_OUT * Wff2, 0.5 * S_OUT * (Wta + Wtb)]  # each [512, 128]
    bj = [bff1, bff2, 0.5 * (bta + btb)]
    wff_arr = np.zeros((128, 12 * 128), np.float32)
    for j in range(3):
        for k in range(NK):
            wff_arr[:, (j * NK + k) * 128 : (j * NK + k + 1) * 128] = Aj[j][
                k * 128 : (k + 1) * 128, :
            ].T

    # bias pack: rows 0-7 = [bff1 k-slices, bff2 k-slices] (bank0, row-group
    # 0), rows 64-67 = tau bias k-slices (bank1, row-group 2).
    biasc_arr = np.zeros((68, 128), np.float32)
    biasc_arr[0:4, :] = bj[0].reshape(4, 128)
    biasc_arr[4:8, :] = bj[1].reshape(4, 128)
    biasc_arr[64:68, :] = bj[2].reshape(4, 128)

    # indicator: rows 0-7 select W-col blocks 0..7 (8W); rows 64-67 select
    # blocks 0..3 (4W). Rows 0-3 double as the y1-head bias indicator.
    indc_arr = np.zeros((68, 8 * W), np.float32)
    for i in range(8):
        indc_arr[i, i * W : (i + 1) * W] = 1.0
    for i in range(4):
        indc_arr[64 + i, i * W : (i + 1) * W] = 1.0

    # fp8 DoubleRow packs: K-row pairs (kp, j) -> virtual row 2*kp+j
    b8_arr = np.zeros((68, 256), np.float32)
    for kp in range(4):
        for j in range(2):
            b8_arr[kp, j * 128 : (j + 1) * 128] = biasc_arr[2 * kp + j]
    for kp in range(2):
        for j in range(2):
            b8_arr[64 + kp, j * 128 : (j + 1) * 128] = biasc_arr[64 + 2 * kp + j]
    i8_arr = np.zeros((68, 1024), np.float32)
    for kp in range(4):
        for j in range(2):
            i8_arr[kp, j * 512 : (j + 1) * 512] = indc_arr[2 * kp + j]
    for kp in range(2):
        for j in range(2):
            i8_arr[64 + kp, j * 256 : (j + 1) * 256] = indc_arr[64 + 2 * kp + j, 0:256]

    wf1_arr = np.zeros((128, 16 * 128), np.float32)
    for m in range(4):
        for k in range(NK):
            wf1_arr[:, (m * NK + k) * 128 : (m * NK + k + 1) * 128] = Wf1[
                m * 128 : (m + 1) * 128, k * 128 : (k + 1) * 128
            ].T
    bf1T_arr = bf1.reshape(4, 128)

    wf2_arr = np.zeros((128, NK * A), np.float32)
    for k in range(NK):
        wf2_arr[:, k * A : (k + 1) * A] = Wf2[:, k * 128 : (k + 1) * 128].T
    bf2_arr = bf2.reshape(1, A)
    ones_arr = np.ones((1, BC), np.float32)

    return {
        "wbbh": _to_bf16(wbbh_arr),
        "wbbhh": _to_bf16(0.5 * wbbh_arr),
        "wbbx": _to_bf16(wbbx_aug),
        "wff": _to_bf16(wff_arr),
        "biasc": _to_bf16(biasc_arr),
        "indc": _to_bf16(indc_arr),
        "wf1": _to_bf16(wf1_arr),
        "bf1T": _to_bf16(bf1T_arr),
        "wf2": _to_bf16(wf2_arr),
        "bf2": _to_bf16(bf2_arr),
        "ones": _to_bf16(ones_arr),
        "b8": np.ascontiguousarray(b8_arr.astype(ml_dtypes.float8_e4m3)),
        "i8": np.ascontiguousarray(i8_arr.astype(ml_dtypes.float8_e4m3)),
    }


def _prep_core_x(state_c):
    # state_c: [BC, T, D] -> xT_aug [D+1, T*BC] with ones row
    xT = state_c.transpose(2, 1, 0).reshape(D, T * BC)  # [d, t*BC+b]
    xa = np.ones((D + 1, T * BC), np.float32)
    xa[:D, :] = xT
    return _to_bf16(xa)


def _get_program():
    if "nc" not in _CACHE:
        _CACHE["nc"] = _build_program()
    return _CACHE["nc"]


def run(inputs, trace=False, trace_kwargs=None):
    inputs = {k: np.asarray(v) for k, v in inputs.items()}
    nc = _get_program()
    shared = _prep_shared(
        inputs["Wbb"], inputs["bbb"], inputs["Wff1"], inputs["bff1"],
        inputs["Wff2"], inputs["bff2"], inputs["Wta"], inputs["bta"],
        inputs["Wtb"], inputs["btb"], inputs["Wf1"], inputs["bf1"],
        inputs["Wf2"], inputs["bf2"],
    )
    state = inputs["state"].astype(np.float32)
    in_maps = []
    for c in range(NCORES):
        m = dict(shared)
        m["xT"] = _prep_core_x(state[c * BC : (c + 1) * BC])
        in_maps.append(m)
    res = run_bass_kernel_spmd(
        nc, in_maps, core_ids=list(range(NCORES)), trace=trace,
        **(trace_kwargs or {}),
    )
    out = np.concatenate(
        [np.asarray(res.results[c]["out"]).T for c in range(NCORES)], axis=0
    )
    return out.astype(np.float32), res


def kernel(**inputs):
    out, _ = run(inputs, trace=False)
    return out


# revision 52
# speedup vs baseline: 1.1973x; 1.0138x over previous
"""CfC (nn_Actor) Trainium2 kernel: 8-core data-parallel recurrent scan.

Layout strategy: everything "transposed" (features on SBUF partitions, batch on
the free dim) so the recurrent state hT feeds the next step's matmuls with zero
data movement. Host-side numpy does all the layout marshalling: x is
pre-transposed with a ones-row appended (folds the backbone bias into the
matmul), weights are pre-transposed/pre-scaled (lecun_tanh scales folded into
weights; sigmoid computed as 0.5*tanh(0.5x)+0.5 with the 0.5 folded in).

ILV independent half-batch scans are interleaved so one scan's matmuls fill
the other's activation/vector-engine bubbles (the step chain is serial).
Per-feature biases enter PSUM via small indicator matmuls packed into
concurrent PE row-groups, scheduled off the critical chain.
"""

import numpy as np
import ml_dtypes

import concourse.bass as bass
import concourse.tile as tile
from concourse import bacc, mybir
from concourse.bass_utils import run_bass_kernel_spmd

BF16 = mybir.dt.bfloat16
F32 = mybir.dt.float32
F8 = mybir.dt.float8e4
TANH = mybir.ActivationFunctionType.Tanh
SIGMOID = mybir.ActivationFunctionType.Sigmoid
ALU = mybir.AluOpType

B, T, D, H, BBD, A = 1024, 256, 64, 512, 128, 8
NCORES = 8
BC = B // NCORES  # batch per core = 128
NK = H // 128  # 4 h-tiles

ILV = 2  # interleaved independent scans per core
RING_EDGES = True  # pin Scalar-engine ring order: big_A, g_B, g_A(t+1), big_B
FP8_BIAS = False  # fp8+DoubleRow biases: no measured speedup (chain-bound), costs accuracy
W = BC // ILV  # batch width per scan
WARMUP_MMS = 0  # PE warm state does not persist for this duty cycle; skip

S_IN = 0.666
S_OUT = 1.7159

_CACHE = {}


def _emit_scan(nc, tc, pools, consts, s):
    """Emit one independent scan over T steps for batch slice s (width W).

    PSUM layout per step (one [128, 1024] tile = 2 banks, double-buffered):
      bank0 (cols 0:512)    = ff1 (4 blocks of W) | ff2 (4 blocks of W)
      bank1 (cols 512:1024) = tau (4 blocks of W) at 512:768, gpre at 768:832
    The two bias matmuls are the per-bank `start=True` cleaners; the gpre
    group writes into bank1's cleared-but-unwritten columns (start=False
    fresh-writes where has_written is unset).
    """
    gpool, ffvpool, hpool, wpool, psg_pool, psf_pool = pools
    xT, wbbh, wbbx, wff, biasc, indc, wf1, bf1T, wf2, bf2, ones = consts[:11]
    wbbhh = consts[11]  # 0.5 * wbbh, for the v-part of the distributed gpre

    GP = 768  # gpre column offset inside the merged PSUM tile
    ff1_prev = None
    v_prev = None
    for t in range(T):
        psf = psf_pool.tile([128, 1024], F32, tag=f"psf{s}")
        # bias writes / bank cleaners (off the critical chain). Two matmuls
        # in distinct PE row-groups (rows 0-7 / 64-67) targeting different
        # banks run concurrently in the array. With FP8_BIAS, fp8e4 operands
        # + DoubleRow pack K-row pairs -> 0.5 cycles/output-row (half time).
        if FP8_BIAS:
            b8, i8 = consts[12], consts[13]
            DR = mybir.MatmulPerfMode.DoubleRow
            nc.tensor.matmul(
                psf[:, 0 : 8 * W],
                b8[0:4, :].rearrange("k (a m) -> k a m", a=2),
                i8[0:4, 0 : 16 * W].rearrange("k (a n) -> k a n", a=2),
                start=True, stop=False, skip_group_check=True, perf_mode=DR,
            )
            # tau bias stays bf16: at N=256 DoubleRow's doubled LDWEIGHTS
            # outweighs the halved streaming (measured 388 -> 600ns)
            taubias = nc.tensor.matmul(
                psf[:, 8 * W : 12 * W], biasc[64:68, :], indc[64:68, 0 : 4 * W],
                start=True, stop=False, skip_group_check=True,
            )
        else:
            nc.tensor.matmul(
                psf[:, 0 : 8 * W], biasc[0:8, :], indc[0:8, 0 : 8 * W],
                start=True, stop=False, skip_group_check=True,
            )
            taubias = nc.tensor.matmul(
                psf[:, 8 * W : 12 * W], biasc[64:68, :], indc[64:68, 0 : 4 * W],
                start=True, stop=False, skip_group_check=True,
            )

        # ---- backbone pre-activation, transposed: [BBD=128, W] ----
        xmm = nc.tensor.matmul(
            psf[:, GP : GP + W], wbbx[:],
            xT[:, t * BC + s * W : t * BC + (s + 1) * W],
            start=False, stop=(t == 0), skip_group_check=True,
        )
        # bank1's cleaner (taubias) must precede the gpre writers
        tile.add_dep_helper(xmm.ins, taubias.ins, sync=False, reason="bank1 clear order")
        if t > 0:
            # gpre = Wbbh@h distributed through h = ff1 + 0.5*v:
            # ff1-part matmuls are ready right after the previous bigACT
            # (they overlap the DVE combine); only the v-part gates on DVE.
            for k in range(NK):
                nc.tensor.matmul(
                    psf[:, GP : GP + W],
                    wbbh[:, k * 128 : (k + 1) * 128],
                    ff1_prev[:, k * W : (k + 1) * W],
                    start=False, stop=False, skip_group_check=True,
                )
            for k in range(NK):
                nc.tensor.matmul(
                    psf[:, GP : GP + W],
                    wbbhh[:, k * 128 : (k + 1) * 128],
                    v_prev[:, k * W : (k + 1) * W],
                    start=False, stop=(k == NK - 1), skip_group_check=True,
                )
        gT = gpool.tile([128, W], BF16, tag=f"gT{s}")
        gact = nc.scalar.activation(gT[:], psf[:, GP : GP + W], TANH)

        # ---- ff1 | ff2 | tau data matmuls ----
        for j in range(3):
            for k in range(NK):
                c0 = (j * NK + k) * W
                nc.tensor.matmul(
                    psf[:, c0 : c0 + W],
                    wff[:, (j * NK + k) * 128 : (j * NK + k + 1) * 128],
                    gT[:],
                    start=False,
                    stop=(j == 2 and k == NK - 1), skip_group_check=True,
                )
        ffv = ffvpool.tile([128, 12 * W], BF16, tag=f"ffv{s}")
        bigact = nc.scalar.activation(ffv[:], psf[:, 0 : 12 * W], TANH)
        ff1v = ffv[:, 0 : 4 * W]
        ff2v = ffv[:, 4 * W : 8 * W]
        tauv = ffv[:, 8 * W : 12 * W]

        # ---- combine: h = ff1 + sig*(ff2-ff1), sig = (tau+1)/2 ----
        # fused: u = ff2-ff1; v = (tau+1)*u; h = 0.5*v + ff1
        u = wpool.tile([128, 4 * W], BF16, tag=f"u{s}")
        uop = nc.vector.tensor_tensor(u[:], ff2v, ff1v, ALU.subtract)
        v = hpool.tile([128, 4 * W], BF16, tag=f"v{s}")
        vop = nc.vector.scalar_tensor_tensor(v[:], tauv, 1.0, u[:], ALU.add, ALU.mult)
        ff1_prev = ff1v
        v_prev = v
        yield (gact, bigact, uop, vop)  # step boundary: interleave scans here

    # ---- head: materialize h = ff1 + 0.5*v once, then y1 = tanh(Wf1@h+bf1)
    h_prev = wpool.tile([128, 4 * W], BF16, tag=f"hl{s}")
    nc.vector.scalar_tensor_tensor(h_prev[:], v_prev[:], 0.5, ff1_prev, ALU.mult, ALU.add)
    psy = psf_pool.tile([128, 4 * W], F32, tag=f"psf{s}")
    nc.tensor.matmul(
        psy[:, 0 : 4 * W], bf1T[0:4, :], indc[0:4, 0 : 4 * W],
        start=True, stop=False,
    )
    for m in range(4):
        for k in range(NK):
            nc.tensor.matmul(
                psy[:, m * W : (m + 1) * W],
                wf1[:, (m * NK + k) * 128 : (m * NK + k + 1) * 128],
                h_prev[:, k * W : (k + 1) * W],
                start=False, stop=(m == 3 and k == NK - 1),
            )
    y1v = ffvpool.tile([128, 4 * W], BF16, tag=f"ffv{s}")
    nc.scalar.activation(y1v[:], psy[:], TANH)

    # ---- head 2: out = tanh(Wf2 @ y1 + bf2) : [A=8, W] ----
    pso = psf_pool.tile([128, W], F32, tag=f"psf{s}")
    nc.tensor.matmul(pso[0:A, :], bf2[0:1, 0:A], ones[0:1, 0:W], start=True, stop=False)
    for k in range(NK):
        nc.tensor.matmul(
            pso[0:A, :],
            wf2[:, k * A : (k + 1) * A],
            y1v[:, k * W : (k + 1) * W],
            start=False, stop=(k == NK - 1),
        )
    out_sb = wpool.tile([128, W], F32, tag=f"osb{s}")
    nc.scalar.activation(out_sb[0:A, :], pso[0:A, :], TANH)
    return out_sb


def _build_program():
    nc = bacc.Bacc("TRN2", target_bir_lowering=False, debug=False)

    xT_e = nc.declare_dram_parameter("xT", [D + 1, T * BC], BF16, isOutput=False)
    wbbh_e = nc.declare_dram_parameter("wbbh", [128, NK * 128], BF16, isOutput=False)
    wbbx_e = nc.declare_dram_parameter("wbbx", [D + 1, 128], BF16, isOutput=False)
    wff_e = nc.declare_dram_parameter("wff", [128, 12 * 128], BF16, isOutput=False)
    biasc_e = nc.declare_dram_parameter("biasc", [68, 128], BF16, isOutput=False)
    indc_e = nc.declare_dram_parameter("indc", [68, 8 * W], BF16, isOutput=False)
    wf1_e = nc.declare_dram_parameter("wf1", [128, 16 * 128], BF16, isOutput=False)
    bf1T_e = nc.declare_dram_parameter("bf1T", [4, 128], BF16, isOutput=False)
    wf2_e = nc.declare_dram_parameter("wf2", [128, NK * A], BF16, isOutput=False)
    bf2_e = nc.declare_dram_parameter("bf2", [1, A], BF16, isOutput=False)
    ones_e = nc.declare_dram_parameter("ones", [1, BC], BF16, isOutput=False)
    wbbhh_e = nc.declare_dram_parameter("wbbhh", [128, NK * 128], BF16, isOutput=False)
    b8_e = nc.declare_dram_parameter("b8", [68, 256], F8, isOutput=False)
    i8_e = nc.declare_dram_parameter("i8", [68, 1024], F8, isOutput=False)
    out_e = nc.declare_dram_parameter("out", [A, BC], F32, isOutput=True)

    from contextlib import ExitStack

    with tile.TileContext(nc) as tc, ExitStack() as ctx:
        const = ctx.enter_context(tc.tile_pool(name="const", bufs=1))
        gpool = ctx.enter_context(tc.tile_pool(name="g", bufs=6))
        ffvpool = ctx.enter_context(tc.tile_pool(name="ffv", bufs=6))
        hpool = ctx.enter_context(tc.tile_pool(name="h", bufs=4))
        wpool = ctx.enter_context(tc.tile_pool(name="work", bufs=4))
        psg_pool = None
        psf_pool = ctx.enter_context(tc.tile_pool(name="psf", bufs=2, space="PSUM"))

        def cload(ext, shape, tag, dtype=BF16):
            t = const.tile(shape, dtype, tag=tag)
            nc.sync.dma_start(t[:], ext[:])
            return t

        consts = (
            cload(xT_e, [D + 1, T * BC], "xT"),
            cload(wbbh_e, [128, NK * 128], "wbbh"),
            cload(wbbx_e, [D + 1, 128], "wbbx"),
            cload(wff_e, [128, 12 * 128], "wff"),
            cload(biasc_e, [68, 128], "biasc"),
            cload(indc_e, [68, 8 * W], "indc"),
            cload(wf1_e, [128, 16 * 128], "wf1"),
            cload(bf1T_e, [4, 128], "bf1T"),
            cload(wf2_e, [128, NK * A], "wf2"),
            cload(bf2_e, [1, A], "bf2"),
            cload(ones_e, [1, BC], "ones"),
            cload(wbbhh_e, [128, NK * 128], "wbbhh"),
            cload(b8_e, [68, 256], "b8", dtype=F8),
            cload(i8_e, [68, 1024], "i8", dtype=F8),
        )
        pools = (gpool, ffvpool, hpool, wpool, psg_pool, psf_pool)

        # PE clock warmup: HAM un-throttles (1.2 -> 2.4 GHz) only after a
        # sustained-busy window; run garbage matmuls while the xT DMA streams.
        wff_c = consts[3]
        warm_ps = psf_pool.tile([128, 512], F32, tag="psf0")
        for _ in range(WARMUP_MMS):
            nc.tensor.matmul(warm_ps[:], wff_c[:, 0:128], wff_c[:, 0:512],
                             start=True, stop=True)

        # Emit the two scans step-interleaved, and pin the Scalar-engine order
        # per period to [big_A, g_B, g_A, big_B]: this keeps each scan's
        # ff-matmul latency and the other scan's bigACT off the critical ring.
        gens = [_emit_scan(nc, tc, pools, consts, s) for s in range(ILV)]
        outs = [None] * ILV
        done = 0
        prev_bB = None
        while done < ILV:
            acts = []
            for s, g in enumerate(gens):
                if outs[s] is None:
                    try:
                        acts.append(next(g))
                    except StopIteration as e:
                        outs[s] = e.value
                        acts.append(None)
                        done += 1
            # Scalar ring per period: big_A(t), g_B(t), g_A(t+1), big_B(t)
            if RING_EDGES and ILV == 2 and len(acts) == 2 and acts[0] and acts[1]:
                (gA, bA, uA, hA), (gB, bB, uB, hB) = acts
                if prev_bB is not None:
                    prev_gB, prev_bB_i, prev_hB = prev_bB
                    tile.add_dep_helper(gA.ins, prev_gB.ins, sync=False,
                                        reason="scalar ring g_A after g_B")
                    tile.add_dep_helper(prev_bB_i.ins, gA.ins, sync=False,
                                        reason="scalar ring big_B after g_A")
                    tile.add_dep_helper(bA.ins, prev_bB_i.ins, sync=False,
                                        reason="scalar ring big_A after big_B")
                    # DVE ring: u_A(t) after h_B(t-1), u_B(t) after h_A(t)
                    tile.add_dep_helper(uA.ins, prev_hB.ins, sync=False,
                                        reason="dve ring u_A after h_B")
                tile.add_dep_helper(gB.ins, bA.ins, sync=False,
                                    reason="scalar ring g_B after big_A")
                tile.add_dep_helper(uB.ins, hA.ins, sync=False,
                                    reason="dve ring u_B after h_A")
                prev_bB = (gB, bB, hB)
        for s, osb in enumerate(outs):
            nc.sync.dma_start(out_e[:, s * W : (s + 1) * W], osb[0:A, :])

    nc.compile()
    return nc


def _to_bf16(x):
    return np.ascontiguousarray(x.astype(ml_dtypes.bfloat16))


def _prep_shared(Wbb, bbb, Wff1, bff1, Wff2, bff2, Wta, bta, Wtb, btb, Wf1, bf1, Wf2, bf2):
    Wbbx = Wbb[:, :D]  # [128, 64]
    Wbbh = Wbb[:, D:]  # [128, 512]

    wbbx_aug = np.zeros((D + 1, 128), np.float32)
    wbbx_aug[:D, :] = (S_IN * Wbbx).T
    wbbx_aug[D, :] = S_IN * bbb

    wbbh_arr = np.zeros((128, NK * 128), np.float32)
    for k in range(NK):
        wbbh_arr[:, k * 128 : (k + 1) * 128] = (S_IN * Wbbh[:, k * 128 : (k + 1) * 128]).T

    Aj = [S_OUT * Wff1, S_OUT * Wff2, 0.5 * S_OUT * (Wta + Wtb)]  # each [512, 128]
    bj = [bff1, bff2, 0.5 * (bta + btb)]
    wff_arr = np.zeros((128, 12 * 128), np.float32)
    for j in range(3):
        for k in range(NK):
            wff_arr[:, (j * NK + k) * 128 : (j * NK + k + 1) * 128] = Aj[j][
                k * 128 : (k + 1) * 128, :
            ].T

    # bias pack: rows 0-7 = [bff1 k-slices, bff2 k-slices] (bank0, row-group
    # 0), rows 64-67 = tau bias k-slices (bank1, row-group 2).
    biasc_arr = np.zeros((68, 128), np.float32)
    biasc_arr[0:4, :] = bj[0].reshape(4, 128)
    biasc_arr[4:8, :] = bj[1].reshape(4, 128)
    biasc_arr[64:68, :] = bj[2].reshape(4, 128)

    # indicator: rows 0-7 select W-col blocks 0..7 (8W); rows 64-67 select
    # blocks 0..3 (4W). Rows 0-3 double as the y1-head bias indicator.
    indc_arr = np.zeros((68, 8 * W), np.float32)
    for i in range(8):
        indc_arr[i, i * W : (i + 1) * W] = 1.0
    for i in range(4):
        indc_arr[64 + i, i * W : (i + 1) * W] = 1.0

    # fp8 DoubleRow packs: K-row pairs (kp, j) -> virtual row 2*kp+j
    b8_arr = np.zeros((68, 256), np.float32)
    for kp in range(4):
        for j in range(2):
            b8_arr[kp, j * 128 : (j + 1) * 128] = biasc_arr[2 * kp + j]
    for kp in range(2):
        for j in range(2):
            b8_arr[64 + kp, j * 128 : (j + 1) * 128] = biasc_arr[64 + 2 * kp + j]
    i8_arr = np.zeros((68, 1024), np.float32)
    for kp in range(4):
        for j in range(2):
            i8_arr[kp, j * 512 : (j + 1) * 512] = indc_arr[2 * kp + j]
    for kp in range(2):
        for j in range(2):
            i8_arr[64 + kp, j * 256 : (j + 1) * 256] = indc_arr[64 + 2 * kp + j, 0:256]

    wf1_arr = np.zeros((128, 16 * 128), np.float32)
    for m in range(4):
        for k in range(NK):
            wf1_arr[:, (m * NK + k) * 128 : (m * NK + k + 1) * 128] = Wf1[
                m * 128 : (m + 1) * 128, k * 128 : (k + 1) * 128
            ].T
    bf1T_arr = bf1.reshape(4, 128)

    wf2_arr = np.zeros((128, NK * A), np.float32)
    for k in range(NK):
        wf2_arr[:, k * A : (k + 1) * A] = Wf2[:, k * 128 : (k + 1) * 128].T
    bf2_arr = bf2.reshape(1, A)
    ones_arr = np.ones((1, BC), np.float32)

    return {
        "wbbh": _to_bf16(wbbh_arr),
        "wbbhh": _to_bf16(0.5 * wbbh_arr),
        "wbbx": _to_bf16(wbbx_aug),
        "wff": _to_bf16(wff_arr),
        "biasc": _to_bf16(biasc_arr),
        "indc": _to_bf16(indc_arr),
        "wf1": _to_bf16(wf1_arr),
        "bf1T": _to_bf16(bf1T_arr),
        "wf2": _to_bf16(wf2_arr),
        "bf2": _to_bf16(bf2_arr),
        "ones": _to_bf16(ones_arr),
        "b8": np.ascontiguousarray(b8_arr.astype(ml_dtypes.float8_e4m3)),
        "i8": np.ascontiguousarray(i8_arr.astype(ml_dtypes.float8_e4m3)),
    }


def _prep_core_x(state_c):
    # state_c: [BC, T, D] -> xT_aug [D+1, T*BC] with ones row
    xT = state_c.transpose(2, 1, 0).reshape(D, T * BC)  # [d, t*BC+b]
    xa = np.ones((D + 1, T * BC), np.float32)
    xa[:D, :] = xT
    return _to_bf16(xa)


def _get_program():
    if "nc" not in _CACHE:
        _CACHE["nc"] = _build_program()
    return _CACHE["nc"]


def run(inputs, trace=False, trace_kwargs=None):
    inputs = {k: np.asarray(v) for k, v in inputs.items()}
    nc = _get_program()
    shared = _prep_shared(
        inputs["Wbb"], inputs["bbb"], inputs["Wff1"], inputs["bff1"],
        inputs["Wff2"], inputs["bff2"], inputs["Wta"], inputs["bta"],
        inputs["Wtb"], inputs["btb"], inputs["Wf1"], inputs["bf1"],
        inputs["Wf2"], inputs["bf2"],
    )
    state = inputs["state"].astype(np.float32)
    in_maps = []
    for c in range(NCORES):
        m = dict(shared)
        m["xT"] = _prep_core_x(state[c * BC : (c + 1) * BC])
        in_maps.append(m)
    res = run_bass_kernel_spmd(
        nc, in_maps, core_ids=list(range(NCORES)), trace=trace,
        **(trace_kwargs or {}),
    )
    out = np.concatenate(
        [np.asarray(res.results[c]["out"]).T for c in range(NCORES)], axis=0
    )
    return out.astype(np.float32), res


def kernel(**inputs):
    out, _ = run(inputs, trace=False)
    return out
